# revision 1
# baseline (speedup 1.0000x reference)
"""Trainium2 Bass kernel for BinaryTokenClassificationModel (segment_reduce).

Reference semantics (B=16, L=2048, H=1024, W=1024):
    src = segment_mean(hidden, source_word_ids)   # [B,W,H]
    tgt = segment_mean(hidden, target_word_ids)   # [B,W,H]
    logits[b,s,t,0] = src[b,s]@w_s + tgt[b,t]@w_t + bias

Key algebraic restructuring: the pooled [B,W,H] tensors are never
materialized.  Because the classifier is linear,
    src_proj[b,s] = segment_mean_s( hidden[b,l] @ w_s )
so we compute per-token scalar dots (fused multiply+reduce on the DVE),
segment-reduce the *scalars* (via tiny one-hot matmuls on the PE, using
the factorization w = 128*q + r), and emit the [W,W] output as an outer
broadcast-sum.

Sharding: data-parallel over batch — 2 examples per NeuronCore on 8 cores.
The tiny classifier weights are replicated.
"""

from contextlib import ExitStack

import ml_dtypes
import numpy as np

import concourse.mybir as mybir
import concourse.tile as tile
from concourse import bacc
from concourse.bass_utils import run_bass_kernel_spmd
from concourse.masks import make_identity

P = 128          # partitions
B = 16           # full batch
NCORES = 8
BLOC = B // NCORES   # batches per core = 2
L = 2048         # tokens
H = 1024         # hidden
W = 1024         # words
Q = W // P       # 8 word chunks
NI = L // P      # 16 token tiles per batch (token l = p*NI + i)
ICH = 4          # token tiles loaded per DMA

F32 = mybir.dt.float32
BF16 = mybir.dt.bfloat16
I32 = mybir.dt.int32

# Compute dtype for the per-token dot products. "bf16" halves both the
# hidden-states DMA traffic and the DVE multiply cost (2x packed mode);
# reductions stay fp32 (DVE/ACT accumulate internally in fp32).
DOT_DTYPE = "bf16"
HDT = BF16 if DOT_DTYPE == "bf16" else F32
# Fraction of dot tiles reduced via the fused DVE op (affine_mul_reduce);
# the rest go DVE-mul + ACT-accumulate to balance engine load.
AMR_PATTERN = 8   # (tile_index % AMR_PATTERN) < AMR_KEEP -> fused DVE path
AMR_KEEP = 3

_CACHE = {}


def _build_module():
    nc = bacc.Bacc(None, target_bir_lowering=False, debug=False)
    names = {}
    with tile.TileContext(nc) as tc, ExitStack() as ctx:
        dram = ctx.enter_context(tc.tile_pool(name="dram", bufs=1, space="DRAM"))
        sb_c = ctx.enter_context(tc.tile_pool(name="const", bufs=1))
        sb_h = ctx.enter_context(tc.tile_pool(name="hid", bufs=4))
        sb_s = ctx.enter_context(tc.tile_pool(name="small", bufs=2))
        sb_o = ctx.enter_context(tc.tile_pool(name="outp", bufs=6))
        ps = ctx.enter_context(tc.tile_pool(name="psum", bufs=2, space="PSUM"))

        hid_d = [dram.tile([L, H], HDT, kind="ExternalInput", name=f"hid{b}")
                 for b in range(BLOC)]
        src_d = [dram.tile([L], I32, kind="ExternalInput", name=f"srcids{b}")
                 for b in range(BLOC)]
        tgt_d = [dram.tile([L], I32, kind="ExternalInput", name=f"tgtids{b}")
                 for b in range(BLOC)]
        w_d = dram.tile([P, 2 * H], HDT, kind="ExternalInput")
        b_d = dram.tile([P, 1], F32, kind="ExternalInput")
        out_d = [dram.tile([W, W], F32, kind="ExternalOutput", name=f"logits{b}")
                 for b in range(BLOC)]

        names["hid"] = [t.name for t in hid_d]
        names["src"] = [t.name for t in src_d]
        names["tgt"] = [t.name for t in tgt_d]
        names["w"] = w_d.name
        names["b"] = b_d.name
        names["out"] = [t.name for t in out_d]

        # ---- constants ----
        w_s = sb_c.tile([P, H], HDT, tag="ws")
        w_t = sb_c.tile([P, H], HDT, tag="wt")
        nc.scalar.dma_start(out=w_s[:], in_=w_d[:, 0:H])
        nc.scalar.dma_start(out=w_t[:], in_=w_d[:, H:2 * H])
        b_bc = sb_c.tile([P, 1], F32, tag="bb")
        nc.scalar.dma_start(out=b_bc[:], in_=b_d[:])

        # iota_r16[p, i, r] = r ; iota_q16[p, i, q] = q  (batched one-hot builds)
        iota_r16 = sb_c.tile([P, NI, P], F32, tag="ior")
        nc.gpsimd.iota(iota_r16[:], pattern=[[0, NI], [1, P]], base=0,
                       channel_multiplier=0, allow_small_or_imprecise_dtypes=True)
        iota_q16 = sb_c.tile([P, NI, Q], F32, tag="ioq")
        nc.gpsimd.iota(iota_q16[:], pattern=[[0, NI], [1, Q]], base=0,
                       channel_multiplier=0, allow_small_or_imprecise_dtypes=True)
        ident = sb_c.tile([P, P], F32, tag="id")
        make_identity(nc, ident[:])
        ones = sb_c.tile([P, P], F32, tag="ones")
        nc.vector.memset(ones[:], 1.0)

        for b in range(BLOC):
            hid_ap = hid_d[b][:].rearrange("(p i) h -> p i h", p=P)  # l = p*NI+i

            # ---- ids -> (q, r) one-hots, before the dot loop ----
            qf = {}
            rf = {}
            for side, ids_dram in (("s", src_d[b]), ("t", tgt_d[b])):
                ids_t = sb_s.tile([P, NI], I32, tag="ids")
                nc.sync.dma_start(out=ids_t[:],
                                  in_=ids_dram[:].rearrange("(p i) -> p i", p=P))
                q_i = sb_s.tile([P, NI], I32, tag="qi")
                r_i = sb_s.tile([P, NI], I32, tag="ri")
                nc.vector.tensor_scalar(out=q_i[:], in0=ids_t[:], scalar1=7,
                                        scalar2=None,
                                        op0=mybir.AluOpType.logical_shift_right)
                nc.vector.tensor_scalar(out=r_i[:], in0=ids_t[:], scalar1=127,
                                        scalar2=None,
                                        op0=mybir.AluOpType.bitwise_and)
                qf[side] = sb_s.tile([P, NI], F32, tag=f"qf{side}", name=f"qf{side}")
                rf[side] = sb_s.tile([P, NI], F32, tag=f"rf{side}", name=f"rf{side}")
                nc.vector.tensor_copy(out=qf[side][:], in_=q_i[:])
                nc.vector.tensor_copy(out=rf[side][:], in_=r_i[:])

            or_all = {}
            mdoq = {}
            segT = {}
            dots = {}
            for side in ("s", "t"):
                or_all[side] = sb_s.tile([P, NI, P], F32, tag=f"orall{side}",
                                         name=f"orall{side}")
                nc.vector.tensor_tensor(
                    out=or_all[side][:], in0=iota_r16[:],
                    in1=rf[side][:].to_broadcast([P, NI, P]),
                    op=mybir.AluOpType.is_equal)
                mdoq[side] = sb_s.tile([P, NI, 2 * Q], F32, tag=f"mdoq{side}",
                                       name=f"mdoq{side}")
                nc.vector.tensor_tensor(
                    out=mdoq[side][:, :, Q:2 * Q], in0=iota_q16[:],
                    in1=qf[side][:].to_broadcast([P, NI, Q]),
                    op=mybir.AluOpType.is_equal)
                segT[side] = ps.tile([2 * Q, P], F32, space="PSUM",
                                     tag=f"segT{side}", name=f"segT{side}")
                dots[side] = sb_s.tile([P, NI], F32, tag=f"dots{side}",
                                       name=f"dots{side}")

            # ---- dots (fused mul+reduce on DVE) + interleaved seg matmuls ----
            for ic in range(NI // ICH):
                ht = sb_h.tile([P, ICH, H], HDT, tag="ht")
                nc.sync.dma_start(out=ht[:], in_=hid_ap[:, ic * ICH:(ic + 1) * ICH, :])
                for k in range(ICH):
                    i = ic * ICH + k
                    for sidx, (side, wt) in enumerate((("t", w_t), ("s", w_s))):
                        scratch = sb_s.tile([P, H], HDT, tag="scr", bufs=6)
                        if HDT is F32 or (2 * i + sidx) % AMR_PATTERN < AMR_KEEP:
                            nc.vector.affine_mul_reduce(
                                out=scratch[:], accum_out=dots[side][:, i:i + 1],
                                in0=ht[:, k, :], in1=wt[:], scale=1.0, bias=0.0)
                        else:
                            nc.vector.tensor_tensor(
                                out=scratch[:], in0=ht[:, k, :], in1=wt[:],
                                op=mybir.AluOpType.mult)
                            scratch2 = sb_s.tile([P, H], HDT, tag="scr2", bufs=6)
                            nc.scalar.activation(
                                out=scratch2[:], in_=scratch[:],
                                func=mybir.ActivationFunctionType.Copy,
                                accum_out=dots[side][:, i:i + 1])
                # this chunk's md columns, then its segment matmuls
                sl = slice(ic * ICH, (ic + 1) * ICH)
                for side in ("s", "t"):
                    nc.vector.tensor_tensor(
                        out=mdoq[side][:, sl, 0:Q], in0=mdoq[side][:, sl, Q:2 * Q],
                        in1=dots[side][:, sl].to_broadcast([P, ICH, Q]),
                        op=mybir.AluOpType.mult)
                    for k in range(ICH):
                        i = ic * ICH + k
                        nc.tensor.matmul(out=segT[side][:],
                                         lhsT=mdoq[side][:, i, :],
                                         rhs=or_all[side][:, i, :],
                                         start=(i == 0), stop=(i == NI - 1))

            # ---- per-side epilogue: transpose back, divide ----
            proj = {}
            for side in ("t", "s"):
                segT_sb = sb_s.tile([2 * Q, P], F32, tag="segTsb", name="segTsb")
                nc.scalar.copy(out=segT_sb[:], in_=segT[side][:])
                seg_ps = ps.tile([P, 2 * Q], F32, space="PSUM", tag="seg",
                                 name="seg")
                nc.tensor.transpose(out=seg_ps[:], in_=segT_sb[:],
                                    identity=ident[0:2 * Q, 0:2 * Q])
                cnt = sb_s.tile([P, Q], F32, tag="cnt")
                nc.vector.tensor_scalar(out=cnt[:], in0=seg_ps[:, Q:2 * Q],
                                        scalar1=1.0, scalar2=None,
                                        op0=mybir.AluOpType.max)
                rec = sb_s.tile([P, Q], F32, tag="rec")
                nc.vector.reciprocal(out=rec[:], in_=cnt[:])
                proj[side] = sb_s.tile([P, Q], F32, tag=f"proj{side}", name=f"proj{side}")
                nc.vector.tensor_tensor(out=proj[side][:], in0=seg_ps[:, 0:Q],
                                        in1=rec[:], op=mybir.AluOpType.mult)

            # fold bias into source projection
            proj_sb = sb_s.tile([P, Q], F32, tag="projsb")
            nc.vector.tensor_scalar(out=proj_sb[:], in0=proj["s"][:],
                                    scalar1=b_bc[:, 0:1], scalar2=None,
                                    op0=mybir.AluOpType.add)

            # ---- broadcast tgt projection to a [P, W] row: tp[p, q*128+r] = proj_t[r, q]
            msel = sb_s.tile([P, W], F32, tag="msel")
            for qb in range(Q):
                nc.vector.tensor_scalar(
                    out=msel[:, qb * P:(qb + 1) * P], in0=ident[:],
                    scalar1=proj["t"][:, qb:qb + 1], scalar2=None,
                    op0=mybir.AluOpType.mult)
            bc_sb = sb_s.tile([P, W], F32, tag="bcsb")
            for half in range(2):
                bc_ps = ps.tile([P, W // 2], F32, space="PSUM", tag="bc")
                nc.tensor.matmul(out=bc_ps[:], lhsT=ones[:],
                                 rhs=msel[:, half * (W // 2):(half + 1) * (W // 2)],
                                 start=True, stop=True)
                nc.scalar.copy(out=bc_sb[:, half * (W // 2):(half + 1) * (W // 2)],
                               in_=bc_ps[:])

            # ---- output tiles: out[j*128+p, t] = proj_s[p, j] + tp[t] ----
            out_ap = out_d[b][:].rearrange("(j p) t -> p j t", p=P)
            for j in range(Q):
                ot = sb_o.tile([P, W], F32, tag="ot")
                if b == BLOC - 1 and j % 2 == 0:
                    # tail batch: split adds across DVE and ACT
                    nc.vector.tensor_scalar(
                        out=ot[:], in0=bc_sb[:], scalar1=proj_sb[:, j:j + 1],
                        scalar2=None, op0=mybir.AluOpType.add)
                else:
                    nc.scalar.add(out=ot[:], in_=bc_sb[:], add=proj_sb[:, j:j + 1])
                nc.scalar.dma_start(out=out_ap[:, j, :], in_=ot[:])

    nc.compile()
    return nc, names


def _get_module():
    if "mod" not in _CACHE:
        _CACHE["mod"] = _build_module()
    return _CACHE["mod"]


def _run(hidden, classifier_w, classifier_b, source_word_ids, target_word_ids,
         **spmd_kwargs):
    nc, names = _get_module()
    hdtype = ml_dtypes.bfloat16 if DOT_DTYPE == "bf16" else np.float32
    hidden = np.ascontiguousarray(hidden).astype(hdtype, copy=False)
    w = np.ascontiguousarray(
        np.broadcast_to(np.asarray(classifier_w, dtype=np.float32)
                        .reshape(1, 2 * H), (P, 2 * H)).astype(hdtype))
    bias = np.ascontiguousarray(
        np.broadcast_to(np.asarray(classifier_b, dtype=np.float32)
                        .reshape(1, 1), (P, 1)))
    src = np.ascontiguousarray(source_word_ids, dtype=np.int32)
    tgt = np.ascontiguousarray(target_word_ids, dtype=np.int32)

    in_maps = []
    for c in range(NCORES):
        m = {names["w"]: w, names["b"]: bias}
        for b in range(BLOC):
            gb = c * BLOC + b
            m[names["hid"][b]] = hidden[gb]
            m[names["src"][b]] = src[gb]
            m[names["tgt"][b]] = tgt[gb]
        in_maps.append(m)

    res = run_bass_kernel_spmd(nc, in_maps, core_ids=list(range(NCORES)),
                               **spmd_kwargs)
    out = np.empty((B, W, W, 1), dtype=np.float32)
    for c in range(NCORES):
        for b in range(BLOC):
            out[c * BLOC + b, :, :, 0] = res.results[c][names["out"][b]]
    return out, res


def kernel(hidden, classifier_w, classifier_b, source_word_ids,
           target_word_ids, num_words):
    out, _ = _run(hidden, classifier_w, classifier_b, source_word_ids,
                  target_word_ids)
    return out



# revision 7
# speedup vs baseline: 1.2738x; 1.2738x over previous
"""Trainium2 Bass kernel for BinaryTokenClassificationModel (segment_reduce).

Reference semantics (B=16, L=2048, H=1024, W=1024):
    src = segment_mean(hidden, source_word_ids)   # [B,W,H]
    tgt = segment_mean(hidden, target_word_ids)   # [B,W,H]
    logits[b,s,t,0] = src[b,s]@w_s + tgt[b,t]@w_t + bias

Linear classifier => project tokens to scalars first:
    dot[l, side] = hidden[l] @ w_side            (PE matmul, H on partitions)
then segment-reduce the scalars via one-hot matmuls (factorize word id as
w = 128*q + r), and emit the [W, W] output as a broadcast outer sum.

Differences vs the previous version: hidden is transposed on the HOST to
[H, L] so the per-token dots run on the tensor engine (w[128h,2].T @
hidT[128h,512l] accumulated over 8 h-chunks) instead of costing ~60us of
DVE+ACT elementwise work; the [2, L] dot rows are transposed back to
token-on-partition layout with 16 tiny PE transposes; one-hots and the
output tiles are bf16 (output upcast to fp32 on the host).

Sharding: data-parallel over batch - 2 examples per NeuronCore on 8 cores.
"""

from contextlib import ExitStack

import ml_dtypes
import numpy as np

import concourse.mybir as mybir
import concourse.tile as tile
from concourse import bacc
from concourse.bass_utils import run_bass_kernel_spmd
from concourse.masks import make_identity

P = 128          # partitions
B = 16           # full batch
NCORES = 8
BLOC = B // NCORES   # batches per core = 2
L = 2048         # tokens
H = 1024         # hidden
W = 1024         # words
Q = W // P       # 8 word chunks (w = q*128 + r)
HC = H // P      # 8 hidden chunks
NI = L // P      # 16 token tiles per batch (token l = i*128 + p)
LT = 4           # dots L-tiles of 512 (PSUM bank size)
LTS = L // LT    # 512

F32 = mybir.dt.float32
BF16 = mybir.dt.bfloat16
I32 = mybir.dt.int32

_CACHE = {}


def _build_module():
    nc = bacc.Bacc(None, target_bir_lowering=False, debug=False)
    names = {}
    with tile.TileContext(nc) as tc, ExitStack() as ctx:
        dram = ctx.enter_context(tc.tile_pool(name="dram", bufs=1, space="DRAM"))
        sb_c = ctx.enter_context(tc.tile_pool(name="const", bufs=1))
        sb_h = ctx.enter_context(tc.tile_pool(name="hid", bufs=6))
        sb_s = ctx.enter_context(tc.tile_pool(name="small", bufs=2))
        sb_o = ctx.enter_context(tc.tile_pool(name="outp", bufs=6))
        ps = ctx.enter_context(tc.tile_pool(name="psum", bufs=1, space="PSUM"))

        hid_d = [dram.tile([H, L], BF16, kind="ExternalInput", name=f"hid{b}")
                 for b in range(BLOC)]
        ids_d = [dram.tile([P, 2, NI], I32, kind="ExternalInput", name=f"ids{b}")
                 for b in range(BLOC)]
        wq_d = dram.tile([P, HC, 2], BF16, kind="ExternalInput")
        b_d = dram.tile([P, 1], F32, kind="ExternalInput")
        out_d = [dram.tile([W, W], BF16, kind="ExternalOutput", name=f"logits{b}")
                 for b in range(BLOC)]

        names["hid"] = [t.name for t in hid_d]
        names["ids"] = [t.name for t in ids_d]
        names["w"] = wq_d.name
        names["b"] = b_d.name
        names["out"] = [t.name for t in out_d]

        # ---- constants ----
        wq_sb = sb_c.tile([P, HC, 2], BF16, tag="wq")
        nc.sync.dma_start(out=wq_sb[:], in_=wq_d[:])
        b_sb = sb_c.tile([P, 1], F32, tag="bb")
        nc.sync.dma_start(out=b_sb[:], in_=b_d[:])

        # iota_r[p, r] = r ; iota_q16[p, i, q] = q   (bf16: values < 256 exact)
        iota_r = sb_c.tile([P, P], BF16, tag="ior")
        nc.gpsimd.iota(iota_r[:], pattern=[[1, P]], base=0,
                       channel_multiplier=0, allow_small_or_imprecise_dtypes=True)
        iota_q16 = sb_c.tile([P, NI, Q], F32, tag="ioq")
        nc.gpsimd.iota(iota_q16[:], pattern=[[0, NI], [1, Q]], base=0,
                       channel_multiplier=0, allow_small_or_imprecise_dtypes=True)
        ident_b = sb_c.tile([P, P], BF16, tag="idb")
        make_identity(nc, ident_b[:])
        identF = sb_c.tile([2 * Q, 2 * Q], F32, tag="idf")
        make_identity(nc, identF[:])
        ones_b = sb_c.tile([P, P], BF16, tag="ones")
        nc.vector.memset(ones_b[:], 1.0)

        for b in range(BLOC):
            # h = c*128 + p  (c = chunk, p = partition)
            hid_ap = hid_d[b][:].rearrange("(c p) l -> p c l", p=P)
            out_ap = out_d[b][:].rearrange("(j p) t -> p j t", p=P)

            # ---- ids -> (q, r) floats, both sides at once ----
            ids_t = sb_s.tile([P, 2, NI], I32, tag="ids")
            nc.sync.dma_start(out=ids_t[:], in_=ids_d[b][:])
            q_i = sb_s.tile([P, 2, NI], I32, tag="qi")
            r_i = sb_s.tile([P, 2, NI], I32, tag="ri")
            nc.vector.tensor_scalar(out=q_i[:], in0=ids_t[:], scalar1=7,
                                    scalar2=None,
                                    op0=mybir.AluOpType.logical_shift_right)
            nc.vector.tensor_scalar(out=r_i[:], in0=ids_t[:], scalar1=127,
                                    scalar2=None,
                                    op0=mybir.AluOpType.bitwise_and)
            qf = sb_s.tile([P, 2, NI], F32, tag="qf")
            rf = sb_s.tile([P, 2, NI], F32, tag="rf")
            nc.vector.tensor_copy(out=qf[:], in_=q_i[:])
            nc.vector.tensor_copy(out=rf[:], in_=r_i[:])

            # ---- one-hots (DVE, bf16) ----
            # or_all[p, s, i, r] = (r == r_id)   -> rhs of the seg matmul
            or_all = sb_s.tile([P, 2, NI, P], BF16, tag="orall")
            for s in range(2):
                for i in range(NI):
                    nc.vector.tensor_scalar(
                        out=or_all[:, s, i, :], in0=iota_r[:],
                        scalar1=rf[:, s, i:i + 1], scalar2=None,
                        op0=mybir.AluOpType.is_equal)
            # mdoq[p, s, i, Q:2Q] = (q == q_id); [.., 0:Q] = that * dot (later)
            mdoq = sb_s.tile([P, 2, NI, 2 * Q], BF16, tag="mdoq")
            for s in range(2):
                nc.vector.tensor_tensor(
                    out=mdoq[:, s, :, Q:2 * Q], in0=iota_q16[:],
                    in1=qf[:, s, :].to_broadcast([P, NI, Q]),
                    op=mybir.AluOpType.is_equal)

            # ---- hidden load + dots on the PE ----
            # dots_ps[lt][m, n] = sum_h wq[h, m] * hidT[h, lt*512+n]
            ht_tiles = []
            for dc in range(HC // 2):
                ht = sb_h.tile([P, 2, L], BF16, tag="ht")
                nc.sync.dma_start(out=ht[:], in_=hid_ap[:, dc * 2:dc * 2 + 2, :])
                ht_tiles.append(ht)
            dots_ps = [ps.tile([2, LTS], F32, space="PSUM", tag="dots", bufs=4,
                               name=f"dots{lt}")
                       for lt in range(LT)]
            for c in range(HC):
                for lt in range(LT):
                    nc.tensor.matmul(out=dots_ps[lt][:],
                                     lhsT=wq_sb[:, c, :],
                                     rhs=ht_tiles[c // 2][:, c % 2,
                                                          lt * LTS:(lt + 1) * LTS],
                                     start=(c == 0), stop=(c == HC - 1))

            # copy dots rows to SBUF (bf16), alternating engines
            dots_row = sb_s.tile([2, L], BF16, tag="drow")
            for lt in range(LT):
                if lt % 2 == 0:
                    nc.vector.tensor_copy(out=dots_row[:, lt * LTS:(lt + 1) * LTS],
                                          in_=dots_ps[lt][:])
                else:
                    nc.scalar.copy(out=dots_row[:, lt * LTS:(lt + 1) * LTS],
                                   in_=dots_ps[lt][:])

            # transpose [2, 128] blocks -> dots_sb[p, i, s] (token l = i*128+p)
            dt_ps = ps.tile([P, NI, 2], BF16, space="PSUM", tag="dt", bufs=1)
            for i in range(NI):
                nc.tensor.transpose(out=dt_ps[:, i, :],
                                    in_=dots_row[:, i * P:(i + 1) * P],
                                    identity=ident_b[0:2, 0:2])
            dots_sb = sb_s.tile([P, NI, 2], BF16, tag="dsb")
            nc.vector.tensor_copy(out=dots_sb[:], in_=dt_ps[:])

            # mdoq[.., 0:Q] = q-onehot * dot
            for s in range(2):
                nc.vector.tensor_tensor(
                    out=mdoq[:, s, :, 0:Q], in0=mdoq[:, s, :, Q:2 * Q],
                    in1=dots_sb[:, :, s].to_broadcast([P, NI, Q]),
                    op=mybir.AluOpType.mult)

            # ---- segment reduction: segT2[qc, s, r] over tokens ----
            segT2 = ps.tile([2 * Q, 2, P], F32, space="PSUM", tag="seg", bufs=1)
            for s in range(2):
                for i in range(NI):
                    nc.tensor.matmul(out=segT2[:, s, :],
                                     lhsT=mdoq[:, s, i, :],
                                     rhs=or_all[:, s, i, :],
                                     start=(i == 0), stop=(i == NI - 1))

            # ---- epilogue: transpose back, divide by counts ----
            segT_sb = sb_s.tile([2 * Q, 2, P], F32, tag="segsb")
            nc.scalar.copy(out=segT_sb[:], in_=segT2[:])
            seg_ps = ps.tile([P, 2, 2 * Q], F32, space="PSUM", tag="segps", bufs=1)
            for s in range(2):
                nc.tensor.transpose(out=seg_ps[:, s, :], in_=segT_sb[:, s, :],
                                    identity=identF[:])
            cnt = sb_s.tile([P, 2, Q], F32, tag="cnt")
            nc.vector.tensor_scalar(out=cnt[:], in0=seg_ps[:, :, Q:2 * Q],
                                    scalar1=1.0, scalar2=None,
                                    op0=mybir.AluOpType.max)
            rec = sb_s.tile([P, 2, Q], F32, tag="rec")
            nc.vector.reciprocal(out=rec[:], in_=cnt[:])
            proj = sb_s.tile([P, 2, Q], F32, tag="proj")
            nc.vector.tensor_tensor(out=proj[:], in0=seg_ps[:, :, 0:Q],
                                    in1=rec[:], op=mybir.AluOpType.mult)
            # source projection + bias (per-partition scalar add)
            projs = sb_s.tile([P, Q], F32, tag="projs")
            nc.vector.tensor_scalar(out=projs[:], in0=proj[:, 0, :],
                                    scalar1=b_sb[:, 0:1], scalar2=None,
                                    op0=mybir.AluOpType.add)

            # ---- broadcast tgt projection to a [P, W] row ----
            # msel[p, qb*128+c] = ident[p, c] * proj_t[p, qb]; column-sum it.
            msel = sb_s.tile([P, W], BF16, tag="msel")
            for qb in range(Q):
                nc.vector.tensor_scalar(
                    out=msel[:, qb * P:(qb + 1) * P], in0=ident_b[:],
                    scalar1=proj[:, 1, qb:qb + 1], scalar2=None,
                    op0=mybir.AluOpType.mult)
            bc_sb = sb_s.tile([P, W], BF16, tag="bcsb")
            for half in range(2):
                bc_ps = ps.tile([P, W // 2], F32, space="PSUM", tag="dots",
                                bufs=4, name=f"bc{half}")
                nc.tensor.matmul(out=bc_ps[:], lhsT=ones_b[:],
                                 rhs=msel[:, half * (W // 2):(half + 1) * (W // 2)],
                                 start=True, stop=True)
                if half == 0:
                    nc.vector.tensor_copy(out=bc_sb[:, 0:W // 2], in_=bc_ps[:])
                else:
                    nc.scalar.copy(out=bc_sb[:, W // 2:W], in_=bc_ps[:])

            # ---- output tiles: out[j*128+p, t] = projs[p, j] + bc[t] ----
            eng = ["v", "a", "v", "g", "v", "a", "v", "a"]
            for j in range(Q):
                ot = sb_o.tile([P, W], BF16, tag="ot")
                e = eng[j]
                if e == "v":
                    nc.vector.tensor_scalar(
                        out=ot[:], in0=bc_sb[:], scalar1=projs[:, j:j + 1],
                        scalar2=None, op0=mybir.AluOpType.add)
                elif e == "g":
                    nc.gpsimd.tensor_scalar(
                        out=ot[:], in0=bc_sb[:], scalar1=projs[:, j:j + 1],
                        scalar2=None, op0=mybir.AluOpType.add)
                else:
                    nc.scalar.add(out=ot[:], in_=bc_sb[:], add=projs[:, j:j + 1])
                nc.scalar.dma_start(out=out_ap[:, j, :], in_=ot[:])

    nc.compile()
    return nc, names


def _get_module():
    if "mod" not in _CACHE:
        _CACHE["mod"] = _build_module()
    return _CACHE["mod"]


def _run(hidden, classifier_w, classifier_b, source_word_ids, target_word_ids,
         **spmd_kwargs):
    nc, names = _get_module()
    bf16 = ml_dtypes.bfloat16
    hidden = np.asarray(hidden, dtype=np.float32)
    # [B, H, L] bf16, host-transposed
    hidT = np.ascontiguousarray(hidden.transpose(0, 2, 1)).astype(bf16)

    w = np.asarray(classifier_w, dtype=np.float32).reshape(2 * H)
    # wq[p, c, s] = w_side_s[c*128 + p]
    wq = np.ascontiguousarray(
        np.stack([w[:H].reshape(HC, P).T, w[H:].reshape(HC, P).T],
                 axis=-1).astype(bf16))
    bias = np.ascontiguousarray(
        np.broadcast_to(np.asarray(classifier_b, dtype=np.float32)
                        .reshape(1, 1), (P, 1)))

    src = np.asarray(source_word_ids, dtype=np.int32)
    tgt = np.asarray(target_word_ids, dtype=np.int32)
    # idsT[b, p, s, i] = ids_side[b, i*128 + p]
    idsT = np.ascontiguousarray(
        np.stack([src.reshape(B, NI, P).transpose(0, 2, 1),
                  tgt.reshape(B, NI, P).transpose(0, 2, 1)], axis=2))

    in_maps = []
    for c in range(NCORES):
        m = {names["w"]: wq, names["b"]: bias}
        for b in range(BLOC):
            gb = c * BLOC + b
            m[names["hid"][b]] = hidT[gb]
            m[names["ids"][b]] = idsT[gb]
        in_maps.append(m)

    res = run_bass_kernel_spmd(nc, in_maps, core_ids=list(range(NCORES)),
                               **spmd_kwargs)
    out = np.empty((B, W, W, 1), dtype=np.float32)
    for c in range(NCORES):
        for b in range(BLOC):
            out[c * BLOC + b, :, :, 0] = np.asarray(
                res.results[c][names["out"][b]], dtype=np.float32)
    return out, res


def kernel(hidden, classifier_w, classifier_b, source_word_ids,
           target_word_ids, num_words):
    out, _ = _run(hidden, classifier_w, classifier_b, source_word_ids,
                  target_word_ids)
    return out


# revision 8
# speedup vs baseline: 1.5374x; 1.2069x over previous
"""Trainium2 Bass kernel for BinaryTokenClassificationModel (segment_reduce).

Reference semantics (B=16, L=2048, H=1024, W=1024):
    src = segment_mean(hidden, source_word_ids)   # [B,W,H]
    tgt = segment_mean(hidden, target_word_ids)   # [B,W,H]
    logits[b,s,t,0] = src[b,s]@w_s + tgt[b,t]@w_t + bias

Linear classifier => project tokens to scalars first:
    dot[l, side] = hidden[l] @ w_side            (PE matmul, H on partitions)
then segment-reduce the scalars via one-hot matmuls (factorize word id as
w = 128*q + r), and emit the [W, W] output as a broadcast outer sum.

Differences vs the previous version: hidden is transposed on the HOST to
[H, L] so the per-token dots run on the tensor engine (w[128h,2].T @
hidT[128h,512l] accumulated over 8 h-chunks) instead of costing ~60us of
DVE+ACT elementwise work; the [2, L] dot rows are transposed back to
token-on-partition layout with 16 tiny PE transposes; one-hots and the
output tiles are bf16 (output upcast to fp32 on the host).

Sharding: data-parallel over batch - 2 examples per NeuronCore on 8 cores.
"""

from contextlib import ExitStack

import ml_dtypes
import numpy as np

import concourse.mybir as mybir
import concourse.tile as tile
from concourse import bacc
from concourse.bass_utils import run_bass_kernel_spmd
from concourse.masks import make_identity

P = 128          # partitions
B = 16           # full batch
NCORES = 8
BLOC = B // NCORES   # batches per core = 2
L = 2048         # tokens
H = 1024         # hidden
W = 1024         # words
Q = W // P       # 8 word chunks (w = q*128 + r)
HC = H // P      # 8 hidden chunks
NI = L // P      # 16 token tiles per batch (token l = i*128 + p)
LT = 4           # dots L-tiles of 512 (PSUM bank size)
LTS = L // LT    # 512

F32 = mybir.dt.float32
BF16 = mybir.dt.bfloat16
I32 = mybir.dt.int32

_CACHE = {}


def _build_module():
    nc = bacc.Bacc(None, target_bir_lowering=False, debug=False)
    names = {}
    with tile.TileContext(nc) as tc, ExitStack() as ctx:
        dram = ctx.enter_context(tc.tile_pool(name="dram", bufs=1, space="DRAM"))
        sb_c = ctx.enter_context(tc.tile_pool(name="const", bufs=1))
        sb_h = ctx.enter_context(tc.tile_pool(name="hid", bufs=6))
        sb_s = ctx.enter_context(tc.tile_pool(name="small", bufs=2))
        sb_o = ctx.enter_context(tc.tile_pool(name="outp", bufs=6))
        ps = ctx.enter_context(tc.tile_pool(name="psum", bufs=1, space="PSUM"))

        hid_d = [dram.tile([H, L], BF16, kind="ExternalInput", name=f"hid{b}")
                 for b in range(BLOC)]
        ids_d = [dram.tile([P, 2, NI], I32, kind="ExternalInput", name=f"ids{b}")
                 for b in range(BLOC)]
        wq_d = dram.tile([P, HC, 2], BF16, kind="ExternalInput")
        b_d = dram.tile([P, 1], F32, kind="ExternalInput")
        out_d = [dram.tile([W, W], BF16, kind="ExternalOutput", name=f"logits{b}")
                 for b in range(BLOC)]

        names["hid"] = [t.name for t in hid_d]
        names["ids"] = [t.name for t in ids_d]
        names["w"] = wq_d.name
        names["b"] = b_d.name
        names["out"] = [t.name for t in out_d]

        # ---- constants (hidden loads issued first for a fast start) ----
        wq_sb = sb_c.tile([P, HC, 2], BF16, tag="wq")
        nc.sync.dma_start(out=wq_sb[:], in_=wq_d[:])
        ht_all = []
        for b in range(BLOC):
            hid_ap = hid_d[b][:].rearrange("(c p) l -> p c l", p=P)
            ht_tiles = []
            for dc in range(HC // 2):
                ht = sb_h.tile([P, 2, L], BF16, tag="ht", name=f"ht{b}_{dc}")
                nc.sync.dma_start(out=ht[:], in_=hid_ap[:, dc * 2:dc * 2 + 2, :])
                ht_tiles.append(ht)
            ht_all.append(ht_tiles)
        b_sb = sb_c.tile([P, 1], F32, tag="bb")
        nc.sync.dma_start(out=b_sb[:], in_=b_d[:])

        # iota_r16[p, i, r] = r ; iota_q16[p, i, q] = q  (bf16: values < 256 exact)
        iota_r16 = sb_c.tile([P, NI, P], BF16, tag="ior")
        nc.gpsimd.iota(iota_r16[:], pattern=[[0, NI], [1, P]], base=0,
                       channel_multiplier=0, allow_small_or_imprecise_dtypes=True)
        iota_q16 = sb_c.tile([P, NI, Q], F32, tag="ioq")
        nc.gpsimd.iota(iota_q16[:], pattern=[[0, NI], [1, Q]], base=0,
                       channel_multiplier=0, allow_small_or_imprecise_dtypes=True)
        ident_b = sb_c.tile([P, P], BF16, tag="idb")
        make_identity(nc, ident_b[:])
        identF = sb_c.tile([2 * Q, 2 * Q], F32, tag="idf")
        make_identity(nc, identF[:])
        ones_b = sb_c.tile([P, P], BF16, tag="ones")
        nc.vector.memset(ones_b[:], 1.0)

        for b in range(BLOC):
            out_ap = out_d[b][:].rearrange("(j p) t -> p j t", p=P)

            # ---- ids -> (q, r) floats, both sides at once ----
            ids_t = sb_s.tile([P, 2, NI], I32, tag="ids")
            nc.sync.dma_start(out=ids_t[:], in_=ids_d[b][:])
            q_i = sb_s.tile([P, 2, NI], I32, tag="qi")
            r_i = sb_s.tile([P, 2, NI], I32, tag="ri")
            nc.vector.tensor_scalar(out=q_i[:], in0=ids_t[:], scalar1=7,
                                    scalar2=None,
                                    op0=mybir.AluOpType.logical_shift_right)
            nc.vector.tensor_scalar(out=r_i[:], in0=ids_t[:], scalar1=127,
                                    scalar2=None,
                                    op0=mybir.AluOpType.bitwise_and)
            qf = sb_s.tile([P, 2, NI], F32, tag="qf")
            rf = sb_s.tile([P, 2, NI], BF16, tag="rf")
            nc.vector.tensor_copy(out=qf[:], in_=q_i[:])
            nc.vector.tensor_copy(out=rf[:], in_=r_i[:])

            # ---- one-hots (DVE, bf16) ----
            # or_all[p, s, i, r] = (r == r_id)   -> rhs of the seg matmul
            or_all = sb_s.tile([P, 2, NI, P], BF16, tag="orall")
            for s in range(2):
                nc.vector.tensor_tensor(
                    out=or_all[:, s, :, :], in0=iota_r16[:],
                    in1=rf[:, s, :].to_broadcast([P, NI, P]),
                    op=mybir.AluOpType.is_equal)
            # mdoq[p, s, i, Q:2Q] = (q == q_id); [.., 0:Q] = that * dot (later)
            mdoq = sb_s.tile([P, 2, NI, 2 * Q], BF16, tag="mdoq")
            for s in range(2):
                nc.vector.tensor_tensor(
                    out=mdoq[:, s, :, Q:2 * Q], in0=iota_q16[:],
                    in1=qf[:, s, :].to_broadcast([P, NI, Q]),
                    op=mybir.AluOpType.is_equal)

            # ---- dots on the PE ----
            # dots_ps[lt][m, n] = sum_h wq[h, m] * hidT[h, lt*512+n]
            ht_tiles = ht_all[b]
            dots_ps = [ps.tile([2, LTS], F32, space="PSUM", tag="dots", bufs=4,
                               name=f"dots{lt}")
                       for lt in range(LT)]
            for c in range(HC):
                for lt in range(LT):
                    nc.tensor.matmul(out=dots_ps[lt][:],
                                     lhsT=wq_sb[:, c, :],
                                     rhs=ht_tiles[c // 2][:, c % 2,
                                                          lt * LTS:(lt + 1) * LTS],
                                     start=(c == 0), stop=(c == HC - 1))

            # copy dots rows to SBUF (bf16), alternating engines
            dots_row = sb_s.tile([2, L], BF16, tag="drow")
            for lt in range(LT):
                if lt % 2 == 0:
                    nc.vector.tensor_copy(out=dots_row[:, lt * LTS:(lt + 1) * LTS],
                                          in_=dots_ps[lt][:])
                else:
                    nc.scalar.copy(out=dots_row[:, lt * LTS:(lt + 1) * LTS],
                                   in_=dots_ps[lt][:])

            # transpose [2, 128] blocks -> dots_sb[p, i, s] (token l = i*128+p)
            dt_ps = ps.tile([P, NI, 2], BF16, space="PSUM", tag="dt", bufs=1)
            for i in range(NI):
                nc.tensor.transpose(out=dt_ps[:, i, :],
                                    in_=dots_row[:, i * P:(i + 1) * P],
                                    identity=ident_b[0:2, 0:2])
            dots_sb = sb_s.tile([P, NI, 2], BF16, tag="dsb")
            nc.vector.tensor_copy(out=dots_sb[:], in_=dt_ps[:])

            # mdoq[.., 0:Q] = q-onehot * dot
            for s in range(2):
                nc.vector.tensor_tensor(
                    out=mdoq[:, s, :, 0:Q], in0=mdoq[:, s, :, Q:2 * Q],
                    in1=dots_sb[:, :, s].to_broadcast([P, NI, Q]),
                    op=mybir.AluOpType.mult)

            # ---- segment reduction: segT2[qc, s, r] over tokens ----
            segT2 = ps.tile([2 * Q, 2, P], F32, space="PSUM", tag="seg", bufs=1)
            for s in range(2):
                for i in range(NI):
                    nc.tensor.matmul(out=segT2[:, s, :],
                                     lhsT=mdoq[:, s, i, :],
                                     rhs=or_all[:, s, i, :],
                                     start=(i == 0), stop=(i == NI - 1))

            # ---- epilogue: transpose back, divide by counts ----
            segT_sb = sb_s.tile([2 * Q, 2, P], F32, tag="segsb")
            nc.scalar.copy(out=segT_sb[:], in_=segT2[:])
            seg_ps = ps.tile([P, 2, 2 * Q], F32, space="PSUM", tag="segps", bufs=1)
            for s in range(2):
                nc.tensor.transpose(out=seg_ps[:, s, :], in_=segT_sb[:, s, :],
                                    identity=identF[:])
            cnt = sb_s.tile([P, 2, Q], F32, tag="cnt")
            nc.vector.tensor_scalar(out=cnt[:], in0=seg_ps[:, :, Q:2 * Q],
                                    scalar1=1.0, scalar2=None,
                                    op0=mybir.AluOpType.max)
            rec = sb_s.tile([P, 2, Q], F32, tag="rec")
            nc.vector.reciprocal(out=rec[:], in_=cnt[:])
            proj = sb_s.tile([P, 2, Q], F32, tag="proj")
            nc.vector.tensor_tensor(out=proj[:], in0=seg_ps[:, :, 0:Q],
                                    in1=rec[:], op=mybir.AluOpType.mult)
            # source projection + bias (per-partition scalar add)
            projs = sb_s.tile([P, Q], F32, tag="projs")
            nc.vector.tensor_scalar(out=projs[:], in0=proj[:, 0, :],
                                    scalar1=b_sb[:, 0:1], scalar2=None,
                                    op0=mybir.AluOpType.add)

            # ---- broadcast tgt projection to a [P, W] row ----
            # msel[p, qb*128+c] = ident[p, c] * proj_t[p, qb]; column-sum it.
            msel = sb_s.tile([P, W], BF16, tag="msel")
            for qb in range(Q):
                nc.vector.tensor_scalar(
                    out=msel[:, qb * P:(qb + 1) * P], in0=ident_b[:],
                    scalar1=proj[:, 1, qb:qb + 1], scalar2=None,
                    op0=mybir.AluOpType.mult)
            bc_sb = sb_s.tile([P, W], BF16, tag="bcsb")
            for half in range(2):
                bc_ps = ps.tile([P, W // 2], F32, space="PSUM", tag="dots",
                                bufs=4, name=f"bc{half}")
                nc.tensor.matmul(out=bc_ps[:], lhsT=ones_b[:],
                                 rhs=msel[:, half * (W // 2):(half + 1) * (W // 2)],
                                 start=True, stop=True)
                if half == 0:
                    nc.vector.tensor_copy(out=bc_sb[:, 0:W // 2], in_=bc_ps[:])
                else:
                    nc.scalar.copy(out=bc_sb[:, W // 2:W], in_=bc_ps[:])

            # ---- output tiles: out[j*128+p, t] = projs[p, j] + bc[t] ----
            eng = ["v", "a", "v", "v", "a", "v", "a", "v"]
            for j in range(Q):
                ot = sb_o.tile([P, W], BF16, tag="ot")
                if eng[j] == "v":
                    nc.vector.tensor_scalar(
                        out=ot[:], in0=bc_sb[:], scalar1=projs[:, j:j + 1],
                        scalar2=None, op0=mybir.AluOpType.add)
                else:
                    nc.scalar.add(out=ot[:], in_=bc_sb[:], add=projs[:, j:j + 1])
                nc.sync.dma_start(out=out_ap[:, j, :], in_=ot[:])

    nc.compile()
    return nc, names


def _get_module():
    if "mod" not in _CACHE:
        _CACHE["mod"] = _build_module()
    return _CACHE["mod"]


def _run(hidden, classifier_w, classifier_b, source_word_ids, target_word_ids,
         **spmd_kwargs):
    nc, names = _get_module()
    bf16 = ml_dtypes.bfloat16
    hidden = np.asarray(hidden, dtype=np.float32)
    # [B, H, L] bf16, host-transposed
    hidT = np.ascontiguousarray(hidden.transpose(0, 2, 1)).astype(bf16)

    w = np.asarray(classifier_w, dtype=np.float32).reshape(2 * H)
    # wq[p, c, s] = w_side_s[c*128 + p]
    wq = np.ascontiguousarray(
        np.stack([w[:H].reshape(HC, P).T, w[H:].reshape(HC, P).T],
                 axis=-1).astype(bf16))
    bias = np.ascontiguousarray(
        np.broadcast_to(np.asarray(classifier_b, dtype=np.float32)
                        .reshape(1, 1), (P, 1)))

    src = np.asarray(source_word_ids, dtype=np.int32)
    tgt = np.asarray(target_word_ids, dtype=np.int32)
    # idsT[b, p, s, i] = ids_side[b, i*128 + p]
    idsT = np.ascontiguousarray(
        np.stack([src.reshape(B, NI, P).transpose(0, 2, 1),
                  tgt.reshape(B, NI, P).transpose(0, 2, 1)], axis=2))

    in_maps = []
    for c in range(NCORES):
        m = {names["w"]: wq, names["b"]: bias}
        for b in range(BLOC):
            gb = c * BLOC + b
            m[names["hid"][b]] = hidT[gb]
            m[names["ids"][b]] = idsT[gb]
        in_maps.append(m)

    res = run_bass_kernel_spmd(nc, in_maps, core_ids=list(range(NCORES)),
                               **spmd_kwargs)
    out = np.empty((B, W, W, 1), dtype=np.float32)
    for c in range(NCORES):
        for b in range(BLOC):
            out[c * BLOC + b, :, :, 0] = np.asarray(
                res.results[c][names["out"][b]], dtype=np.float32)
    return out, res


def kernel(hidden, classifier_w, classifier_b, source_word_ids,
           target_word_ids, num_words):
    out, _ = _run(hidden, classifier_w, classifier_b, source_word_ids,
                  target_word_ids)
    return out


# revision 9
# speedup vs baseline: 2.1300x; 1.3855x over previous
"""Trainium2 Bass kernel for BinaryTokenClassificationModel (segment_reduce).

Reference semantics (B=16, L=2048, H=1024, W=1024):
    src = segment_mean(hidden, source_word_ids)   # [B,W,H]
    tgt = segment_mean(hidden, target_word_ids)   # [B,W,H]
    logits[b,s,t,0] = src[b,s]@w_s + tgt[b,t]@w_t + bias

Linear classifier => project tokens to scalars first:
    dot[l, side] = hidden[l] @ w_side            (PE matmul, H on partitions)
then segment-reduce the scalars via one-hot matmuls (factorize word id as
w = 128*q + r), and emit the [W, W] output as a broadcast outer sum.

Differences vs the previous version: hidden is transposed on the HOST to
[H, L] so the per-token dots run on the tensor engine (w[128h,2].T @
hidT[128h,512l] accumulated over 8 h-chunks) instead of costing ~60us of
DVE+ACT elementwise work; the [2, L] dot rows are transposed back to
token-on-partition layout with 16 tiny PE transposes; one-hots and the
output tiles are bf16 (output upcast to fp32 on the host).

Sharding: data-parallel over batch - 2 examples per NeuronCore on 8 cores.
"""

from contextlib import ExitStack

import ml_dtypes
import numpy as np

import concourse.mybir as mybir
import concourse.tile as tile
from concourse import bacc
from concourse.bass_utils import run_bass_kernel_spmd
from concourse.masks import make_identity

P = 128          # partitions
B = 16           # full batch
NCORES = 8
BLOC = B // NCORES   # batches per core = 2
L = 2048         # tokens
H = 1024         # hidden
W = 1024         # words
Q = W // P       # 8 word chunks (w = q*128 + r)
HC = H // P      # 8 hidden chunks
NI = L // P      # 16 token tiles per batch (token l = i*128 + p)
LT = 4           # dots L-tiles of 512 (PSUM bank size)
LTS = L // LT    # 512

F32 = mybir.dt.float32
BF16 = mybir.dt.bfloat16
I32 = mybir.dt.int32

_CACHE = {}


def _build_module():
    nc = bacc.Bacc(None, target_bir_lowering=False, debug=False)
    names = {}
    with tile.TileContext(nc) as tc, ExitStack() as ctx:
        dram = ctx.enter_context(tc.tile_pool(name="dram", bufs=1, space="DRAM"))
        sb_c = ctx.enter_context(tc.tile_pool(name="const", bufs=1))
        sb_h = ctx.enter_context(tc.tile_pool(name="hid", bufs=8))
        sb_s = ctx.enter_context(tc.tile_pool(name="small", bufs=2))
        sb_o = ctx.enter_context(tc.tile_pool(name="outp", bufs=6))
        ps = ctx.enter_context(tc.tile_pool(name="psum", bufs=1, space="PSUM"))

        hid_d = [dram.tile([H, L], BF16, kind="ExternalInput", name=f"hid{b}")
                 for b in range(BLOC)]
        ids_d = [dram.tile([P, 2, NI], I32, kind="ExternalInput", name=f"ids{b}")
                 for b in range(BLOC)]
        wq_d = dram.tile([P, HC, 2], BF16, kind="ExternalInput")
        b_d = dram.tile([P, 1], F32, kind="ExternalInput")
        out_d = [dram.tile([W, W], BF16, kind="ExternalOutput", name=f"logits{b}")
                 for b in range(BLOC)]

        names["hid"] = [t.name for t in hid_d]
        names["ids"] = [t.name for t in ids_d]
        names["w"] = wq_d.name
        names["b"] = b_d.name
        names["out"] = [t.name for t in out_d]

        # ---- constants (ids + hidden loads issued first for a fast start) ----
        wq_sb = sb_c.tile([P, HC, 2], BF16, tag="wq")
        nc.sync.dma_start(out=wq_sb[:], in_=wq_d[:])
        ids_all = []
        for b in range(BLOC):
            ids_t = sb_s.tile([P, 2, NI], I32, tag="ids", name=f"ids_t{b}")
            nc.sync.dma_start(out=ids_t[:], in_=ids_d[b][:])
            ids_all.append(ids_t)
        ht_all = []
        for b in range(BLOC):
            hid_ap = hid_d[b][:].rearrange("(c p) l -> p c l", p=P)
            ht_tiles = []
            for dc in range(HC // 2):
                ht = sb_h.tile([P, 2, L], BF16, tag="ht", name=f"ht{b}_{dc}")
                nc.sync.dma_start(out=ht[:], in_=hid_ap[:, dc * 2:dc * 2 + 2, :])
                ht_tiles.append(ht)
            ht_all.append(ht_tiles)
        b_sb = sb_c.tile([P, 1], F32, tag="bb")
        nc.sync.dma_start(out=b_sb[:], in_=b_d[:])

        # iota_r16[p, i, r] = r ; iota_q16[p, i, q] = q  (bf16: values < 256 exact)
        iota_r16 = sb_c.tile([P, NI, P], BF16, tag="ior")
        nc.gpsimd.iota(iota_r16[:], pattern=[[0, NI], [1, P]], base=0,
                       channel_multiplier=0, allow_small_or_imprecise_dtypes=True)
        iota_q16 = sb_c.tile([P, NI, Q], F32, tag="ioq")
        nc.gpsimd.iota(iota_q16[:], pattern=[[0, NI], [1, Q]], base=0,
                       channel_multiplier=0, allow_small_or_imprecise_dtypes=True)
        ident_b = sb_c.tile([P, P], BF16, tag="idb")
        make_identity(nc, ident_b[:])
        ones_b = sb_c.tile([P, P], BF16, tag="ones")
        nc.vector.memset(ones_b[:], 1.0)

        for b in range(BLOC):
            out_ap = out_d[b][:].rearrange("(j p) t -> p j t", p=P)

            # ---- ids -> (q, r) floats, both sides at once ----
            ids_t = ids_all[b]
            q_i = sb_s.tile([P, 2, NI], I32, tag="qi")
            r_i = sb_s.tile([P, 2, NI], I32, tag="ri")
            nc.vector.tensor_scalar(out=q_i[:], in0=ids_t[:], scalar1=7,
                                    scalar2=None,
                                    op0=mybir.AluOpType.logical_shift_right)
            nc.vector.tensor_scalar(out=r_i[:], in0=ids_t[:], scalar1=127,
                                    scalar2=None,
                                    op0=mybir.AluOpType.bitwise_and)
            qf = sb_s.tile([P, 2, NI], F32, tag="qf")
            rf = sb_s.tile([P, 2, NI], BF16, tag="rf")
            nc.vector.tensor_copy(out=qf[:], in_=q_i[:])
            nc.vector.tensor_copy(out=rf[:], in_=r_i[:])

            # ---- one-hots (DVE, bf16) ----
            # or_all[p, s, i, r] = (r == r_id)   -> rhs of the seg matmul
            or_all = sb_s.tile([P, 2, NI, P], BF16, tag="orall")
            for s in range(2):
                nc.vector.tensor_tensor(
                    out=or_all[:, s, :, :], in0=iota_r16[:],
                    in1=rf[:, s, :].to_broadcast([P, NI, P]),
                    op=mybir.AluOpType.is_equal)
            # mdoq[p, s, i, Q:2Q] = (q == q_id); [.., 0:Q] = that * dot (later)
            mdoq = sb_s.tile([P, 2, NI, 2 * Q], BF16, tag="mdoq")
            for s in range(2):
                nc.vector.tensor_tensor(
                    out=mdoq[:, s, :, Q:2 * Q], in0=iota_q16[:],
                    in1=qf[:, s, :].to_broadcast([P, NI, Q]),
                    op=mybir.AluOpType.is_equal)

            # ---- dots on the PE ----
            # dots_ps[lt][m, n] = sum_h wq[h, m] * hidT[h, lt*512+n]
            ht_tiles = ht_all[b]
            dots_ps = [ps.tile([2, LTS], F32, space="PSUM", tag="dots", bufs=4,
                               name=f"dots{lt}")
                       for lt in range(LT)]
            for c in range(HC):
                for lt in range(LT):
                    nc.tensor.matmul(out=dots_ps[lt][:],
                                     lhsT=wq_sb[:, c, :],
                                     rhs=ht_tiles[c // 2][:, c % 2,
                                                          lt * LTS:(lt + 1) * LTS],
                                     start=(c == 0), stop=(c == HC - 1))

            # copy dots rows to SBUF (bf16), alternating engines
            dots_row = sb_s.tile([2, L], BF16, tag="drow")
            for lt in range(LT):
                if lt % 2 == 0:
                    nc.vector.tensor_copy(out=dots_row[:, lt * LTS:(lt + 1) * LTS],
                                          in_=dots_ps[lt][:])
                else:
                    nc.scalar.copy(out=dots_row[:, lt * LTS:(lt + 1) * LTS],
                                   in_=dots_ps[lt][:])

            # transpose [2, 128] blocks -> dots_sb[p, i, s] (token l = i*128+p)
            dt_ps = ps.tile([P, NI, 2], BF16, space="PSUM", tag="dt", bufs=1)
            for i in range(NI):
                nc.tensor.transpose(out=dt_ps[:, i, :],
                                    in_=dots_row[:, i * P:(i + 1) * P],
                                    identity=ident_b[0:2, 0:2])
            dots_sb = sb_s.tile([P, NI, 2], BF16, tag="dsb")
            nc.vector.tensor_copy(out=dots_sb[:], in_=dt_ps[:])

            # mdoq[.., 0:Q] = q-onehot * dot
            for s in range(2):
                nc.vector.tensor_tensor(
                    out=mdoq[:, s, :, 0:Q], in0=mdoq[:, s, :, Q:2 * Q],
                    in1=dots_sb[:, :, s].to_broadcast([P, NI, Q]),
                    op=mybir.AluOpType.mult)

            # ---- segment reduction, directly in [r, side, 2Q] layout ----
            # seg_ps[r, s, qc] = sum_{p,i} or_all[p, s, i, r] * mdoq[p, s, i, qc]
            seg_ps = ps.tile([P, 2, 2 * Q], F32, space="PSUM", tag="segps", bufs=1)
            for s in range(2):
                for i in range(NI):
                    nc.tensor.matmul(out=seg_ps[:, s, :],
                                     lhsT=or_all[:, s, i, :],
                                     rhs=mdoq[:, s, i, :],
                                     start=(i == 0), stop=(i == NI - 1))

            # ---- epilogue: divide by counts ----
            cnt = sb_s.tile([P, 2, Q], F32, tag="cnt")
            nc.vector.tensor_scalar(out=cnt[:], in0=seg_ps[:, :, Q:2 * Q],
                                    scalar1=1.0, scalar2=None,
                                    op0=mybir.AluOpType.max)
            rec = sb_s.tile([P, 2, Q], F32, tag="rec")
            nc.vector.reciprocal(out=rec[:], in_=cnt[:])
            proj = sb_s.tile([P, 2, Q], F32, tag="proj")
            nc.vector.tensor_tensor(out=proj[:], in0=seg_ps[:, :, 0:Q],
                                    in1=rec[:], op=mybir.AluOpType.mult)
            # source projection + bias (per-partition scalar add)
            projs = sb_s.tile([P, Q], F32, tag="projs")
            nc.vector.tensor_scalar(out=projs[:], in0=proj[:, 0, :],
                                    scalar1=b_sb[:, 0:1], scalar2=None,
                                    op0=mybir.AluOpType.add)

            # ---- broadcast tgt projection to a [P, W] row ----
            # msel[p, qb*128+c] = ident[p, c] * proj_t[p, qb]; column-sum it.
            msel = sb_s.tile([P, W], BF16, tag="msel")
            for qb in range(Q):
                nc.vector.tensor_scalar(
                    out=msel[:, qb * P:(qb + 1) * P], in0=ident_b[:],
                    scalar1=proj[:, 1, qb:qb + 1], scalar2=None,
                    op0=mybir.AluOpType.mult)
            bc_sb = sb_s.tile([P, W], BF16, tag="bcsb")
            for half in range(2):
                bc_ps = ps.tile([P, W // 2], F32, space="PSUM", tag="dots",
                                bufs=4, name=f"bc{half}")
                nc.tensor.matmul(out=bc_ps[:], lhsT=ones_b[:],
                                 rhs=msel[:, half * (W // 2):(half + 1) * (W // 2)],
                                 start=True, stop=True)
                if half == 0:
                    nc.vector.tensor_copy(out=bc_sb[:, 0:W // 2], in_=bc_ps[:])
                else:
                    nc.scalar.copy(out=bc_sb[:, W // 2:W], in_=bc_ps[:])

            # ---- output tiles: out[j*128+p, t] = projs[p, j] + bc[t] ----
            eng = ["v", "a", "v", "a", "v", "a", "v", "v"]
            for jp in range(Q // 2):
                ot = sb_o.tile([P, 2, W], BF16, tag="ot", bufs=4)
                for k in range(2):
                    j = jp * 2 + k
                    if eng[j] == "v":
                        nc.vector.tensor_scalar(
                            out=ot[:, k, :], in0=bc_sb[:],
                            scalar1=projs[:, j:j + 1],
                            scalar2=None, op0=mybir.AluOpType.add)
                    else:
                        nc.scalar.add(out=ot[:, k, :], in_=bc_sb[:],
                                      add=projs[:, j:j + 1])
                dma_eng = nc.sync if jp % 2 == 0 else nc.scalar
                dma_eng.dma_start(out=out_ap[:, jp * 2:jp * 2 + 2, :], in_=ot[:])

    nc.compile()
    return nc, names


def _get_module():
    if "mod" not in _CACHE:
        _CACHE["mod"] = _build_module()
    return _CACHE["mod"]


def _run(hidden, classifier_w, classifier_b, source_word_ids, target_word_ids,
         **spmd_kwargs):
    nc, names = _get_module()
    bf16 = ml_dtypes.bfloat16
    hidden = np.asarray(hidden, dtype=np.float32)
    # [B, H, L] bf16, host-transposed
    hidT = np.ascontiguousarray(hidden.transpose(0, 2, 1)).astype(bf16)

    w = np.asarray(classifier_w, dtype=np.float32).reshape(2 * H)
    # wq[p, c, s] = w_side_s[c*128 + p]
    wq = np.ascontiguousarray(
        np.stack([w[:H].reshape(HC, P).T, w[H:].reshape(HC, P).T],
                 axis=-1).astype(bf16))
    bias = np.ascontiguousarray(
        np.broadcast_to(np.asarray(classifier_b, dtype=np.float32)
                        .reshape(1, 1), (P, 1)))

    src = np.asarray(source_word_ids, dtype=np.int32)
    tgt = np.asarray(target_word_ids, dtype=np.int32)
    # idsT[b, p, s, i] = ids_side[b, i*128 + p]
    idsT = np.ascontiguousarray(
        np.stack([src.reshape(B, NI, P).transpose(0, 2, 1),
                  tgt.reshape(B, NI, P).transpose(0, 2, 1)], axis=2))

    in_maps = []
    for c in range(NCORES):
        m = {names["w"]: wq, names["b"]: bias}
        for b in range(BLOC):
            gb = c * BLOC + b
            m[names["hid"][b]] = hidT[gb]
            m[names["ids"][b]] = idsT[gb]
        in_maps.append(m)

    res = run_bass_kernel_spmd(nc, in_maps, core_ids=list(range(NCORES)),
                               **spmd_kwargs)
    out = np.empty((B, W, W, 1), dtype=np.float32)
    for c in range(NCORES):
        for b in range(BLOC):
            out[c * BLOC + b, :, :, 0] = np.asarray(
                res.results[c][names["out"][b]], dtype=np.float32)
    return out, res


def kernel(hidden, classifier_w, classifier_b, source_word_ids,
           target_word_ids, num_words):
    out, _ = _run(hidden, classifier_w, classifier_b, source_word_ids,
                  target_word_ids)
    return out
